# revision 1
# baseline (speedup 1.0000x reference)
"""Trainium2 Bass kernel for the BatchedEdges GNN message-passing module.

Strategy (edge-parallel over 8 NeuronCores):
  - The E=4032 directed edges are dealt to the 8 cores balanced by target
    region, grouped into runs of GS=8 slots sharing one target (padded with
    zero-weight dummy slots).
  - Per slot (edge) on device:
      GEMM1: psum[128,16] = W1[e]^T-slab (stationary, [S=128, 2M=128]) @
             src[e]^T ([S=128, B=16]); rows of the output are [meanT;logstdT]
             (order swapped for odd slots).
      bias:  DVE per-partition bias add, PSUM -> SBUF staging tile.
      GEMM2: one matmul per slot, stationary msgT=meanT+bias ([M=64,B=16],
             partitions 0-63 for even slots / 64-127 for odd slots), moving
             operand the fused [addW^T | gainW^T] slab ([64, 256]);
             accumulates over the group's slots into a per-parity PSUM
             accumulator (PE PSUM accumulation).
  - Group epilogue: scalar-engine copies of the two accumulators to SBUF,
    DMA out; the host adds the parities and scatter-adds groups->targets.
  - All weight slabs are pre-transposed/interleaved on the host so every DMA
    lands on 128 SBUF partitions with >=512B contiguous per partition row.

The host does gather (source rows per edge), all layout packing, the final
scatter into the dense [B,R,R,M] outputs, and the 8-way partial reduction of
the [B,R,L] incoming-message sums.
"""

import numpy as np

B, R, S, M, L = 16, 64, 128, 64, 128
NC = 8
GS = 8  # slots per PSUM accumulation group (one target per group)

_F32 = np.float32
_prog_cache = {}


def _split_excess_waits(nc, cap=1):
    """This container's walrus rejects >1 semaphore wait per instruction;
    move excess waits onto preceding same-engine no-ops."""
    import concourse.mybir as mybir

    for fn in nc.m.functions:
        for blk in fn.blocks:
            insts = list(blk.instructions)
            out = []
            changed = False
            for inst in insts:
                si = getattr(inst, "sync_info", None)
                if si is not None and si.on_wait is not None and len(si.on_wait) > cap:
                    waits = list(si.on_wait)
                    excess, keep = waits[:-cap], waits[-cap:]
                    for ci in range(0, len(excess), cap):
                        out.append(
                            mybir.InstNoOp(
                                name=f"{inst.name}-wsplit{ci}",
                                engine=inst.engine,
                                ins=[],
                                outs=[],
                                sync_info=mybir.SyncInfo(
                                    on_wait=excess[ci : ci + cap], on_update=[]
                                ),
                            )
                        )
                    si.on_wait = keep
                    changed = True
                out.append(inst)
            if changed:
                blk.instructions = out


def _build_program(ng, bufs=4):
    import concourse.bass as bass
    import concourse.tile as tile
    from concourse import mybir

    f32 = mybir.dt.float32
    nslot = ng * GS
    nc = bass.Bass()
    w1 = nc.dram_tensor("w1", [S, nslot, 2 * M], f32, kind="ExternalInput")
    ag = nc.dram_tensor("ag", [2 * M, nslot // 2, 2 * L], f32, kind="ExternalInput")
    sg = nc.dram_tensor("sg", [S, nslot, B], f32, kind="ExternalInput")
    bi = nc.dram_tensor("bi", [2 * M, nslot], f32, kind="ExternalInput")
    mo = nc.dram_tensor("mo", [2 * M, nslot, B], f32, kind="ExternalOutput")
    io = nc.dram_tensor("io", [B, ng, 2, 2 * L], f32, kind="ExternalOutput")

    act_copy = mybir.ActivationFunctionType.Copy

    with tile.TileContext(nc) as tc:
        with (
            tc.tile_pool(name="wp", bufs=bufs) as wp,
            tc.tile_pool(name="sp", bufs=bufs) as sp,
            tc.tile_pool(name="op", bufs=3) as op,
            tc.tile_pool(name="cp", bufs=1) as cp,
            tc.tile_pool(name="ps1p", bufs=3, space="PSUM") as ps1p,
            tc.tile_pool(name="ps2p", bufs=2, space="PSUM") as ps2p,
        ):
            bit = cp.tile([2 * M, nslot], f32, tag="bit")
            nc.sync.dma_start(bit[:], bi[:])
            for g in range(ng):
                gsl = slice(g * GS, (g + 1) * GS)
                hsl = slice(g * (GS // 2), (g + 1) * (GS // 2))
                w1t = wp.tile([S, GS, 2 * M], f32, tag="w1t")
                agt = wp.tile([2 * M, GS // 2, 2 * L], f32, tag="agt")
                sgt = sp.tile([S, GS, B], f32, tag="sgt")
                ott = op.tile([2 * M, GS, B], f32, tag="ott")
                incst = op.tile([B, 2, 2 * L], f32, tag="incst")
                ps1 = ps1p.tile([S, GS, B], f32, tag="ps1")
                ps2e = ps2p.tile([B, 2 * L], f32, tag="ps2e")
                ps2o = ps2p.tile([B, 2 * L], f32, tag="ps2o")
                nc.sync.dma_start(w1t[:], w1[:, gsl, :])
                nc.sync.dma_start(agt[:], ag[:, hsl, :])
                nc.sync.dma_start(sgt[:], sg[:, gsl, :])
                for i in range(GS):
                    j = g * GS + i
                    nc.tensor.matmul(
                        ps1[:, i, :], w1t[:, i, :], sgt[:, i, :], start=True, stop=True
                    )
                    nc.vector.tensor_scalar_add(
                        ott[:, i, :], ps1[:, i, :], bit[:, j : j + 1]
                    )
                    half = slice(0, M) if i % 2 == 0 else slice(M, 2 * M)
                    tgt = ps2e if i % 2 == 0 else ps2o
                    nc.tensor.matmul(
                        tgt[:],
                        ott[half, i, :],
                        agt[half, i // 2, :],
                        start=(i < 2),
                        stop=(i >= GS - 2),
                    )
                nc.scalar.activation(incst[:, 0, :], ps2e[:], act_copy)
                nc.scalar.activation(incst[:, 1, :], ps2o[:], act_copy)
                nc.sync.dma_start(mo[:, gsl, :], ott[:])
                nc.sync.dma_start(io[:, g, :, :], incst[:])

    _split_excess_waits(nc)
    return nc


def _get_program(ng):
    if ng not in _prog_cache:
        _prog_cache[ng] = _build_program(ng)
    return _prog_cache[ng]


def _plan(src_idx, tgt_idx):
    """Deal edges to cores balanced per target; group into GS-slot runs."""
    e = len(tgt_idx)
    order = np.argsort(tgt_idx, kind="stable")
    sorted_tgt = tgt_idx[order]
    bounds = np.searchsorted(sorted_tgt, np.arange(R + 1))
    core_slots = [[] for _ in range(NC)]
    core_gtgt = [[] for _ in range(NC)]
    for t in range(R):
        eds = order[bounds[t] : bounds[t + 1]]
        if len(eds) == 0:
            continue
        for k in range(NC):
            chunk = eds[(k + t) % NC :: NC]
            if len(chunk) == 0:
                continue
            npad = (-len(chunk)) % GS
            core_slots[k].extend(chunk.tolist())
            core_slots[k].extend([-1] * npad)
            core_gtgt[k].extend([t] * ((len(chunk) + npad) // GS))
    ng = max(len(gt) for gt in core_gtgt)
    for k in range(NC):
        pad_groups = ng - len(core_gtgt[k])
        core_slots[k].extend([-1] * (pad_groups * GS))
        core_gtgt[k].extend([-1] * pad_groups)
    slots = [np.asarray(s, np.int64) for s in core_slots]
    gtgt = [np.asarray(t, np.int64) for t in core_gtgt]
    return ng, slots, gtgt


def kernel(**inputs):
    source = np.ascontiguousarray(np.asarray(inputs["source"], _F32))
    mean_w = np.asarray(inputs["mean_w"], _F32)
    mean_b = np.asarray(inputs["mean_b"], _F32)
    logstd_w = np.asarray(inputs["logstd_w"], _F32)
    logstd_b = np.asarray(inputs["logstd_b"], _F32)
    add_w = np.asarray(inputs["add_w"], _F32)
    gain_w = np.asarray(inputs["gain_w"], _F32)
    src_idx = np.asarray(inputs["src_idx"]).astype(np.int64).ravel()
    tgt_idx = np.asarray(inputs["tgt_idx"]).astype(np.int64).ravel()

    ng, slots, gtgt = _plan(src_idx, tgt_idx)
    nslot = ng * GS
    srcT = np.ascontiguousarray(source.transpose(2, 1, 0))  # [S, R, B]

    in_maps = []
    for k in range(NC):
        sl = slots[k]
        real = sl >= 0
        jr = np.nonzero(real)[0]
        par = jr % 2
        je, jo = jr[par == 0], jr[par == 1]
        ide, ido = sl[je], sl[jo]

        w1 = np.zeros((S, nslot, 2 * M), _F32)
        w1[:, je, 0:M] = mean_w[ide].transpose(2, 0, 1)
        w1[:, je, M:] = logstd_w[ide].transpose(2, 0, 1)
        w1[:, jo, 0:M] = logstd_w[ido].transpose(2, 0, 1)
        w1[:, jo, M:] = mean_w[ido].transpose(2, 0, 1)

        ag = np.zeros((2 * M, nslot // 2, 2 * L), _F32)
        ag[0:M, je // 2, 0:L] = add_w[ide].transpose(2, 0, 1)
        ag[0:M, je // 2, L:] = gain_w[ide].transpose(2, 0, 1)
        ag[M:, jo // 2, 0:L] = add_w[ido].transpose(2, 0, 1)
        ag[M:, jo // 2, L:] = gain_w[ido].transpose(2, 0, 1)

        sg = np.zeros((S, nslot, B), _F32)
        sg[:, jr, :] = srcT[:, src_idx[sl[jr]], :]

        bi = np.zeros((2 * M, nslot), _F32)
        bi[0:M, je] = mean_b[ide].T
        bi[M:, je] = logstd_b[ide].T
        bi[0:M, jo] = logstd_b[ido].T
        bi[M:, jo] = mean_b[ido].T

        in_maps.append({"w1": w1, "ag": ag, "sg": sg, "bi": bi})

    from concourse.bass_utils import run_bass_kernel_spmd

    nc = _get_program(ng)
    res = run_bass_kernel_spmd(nc, in_maps, list(range(NC)))

    inc_addT = np.zeros((R, B, L), _F32)
    inc_gainT = np.zeros((R, B, L), _F32)
    scat_mean = np.zeros((B, R * R, M), _F32)
    scat_logstd = np.zeros((B, R * R, M), _F32)
    for k in range(NC):
        out = res.results[k]
        mo = out["mo"]  # [2M, nslot, B]
        io = out["io"]  # [B, ng, 2, 2L]
        gt = gtgt[k]
        valid = gt >= 0
        gsum = io[:, valid, 0, :] + io[:, valid, 1, :]  # [B, ngv, 2L]
        tv = gt[valid]
        np.add.at(inc_addT, tv, gsum[:, :, 0:L].transpose(1, 0, 2))
        np.add.at(inc_gainT, tv, gsum[:, :, L:].transpose(1, 0, 2))

        sl = slots[k]
        real = sl >= 0
        jr = np.nonzero(real)[0]
        eids = sl[jr]
        flat = src_idx[eids] * R + tgt_idx[eids]
        even = jr % 2 == 0
        meanT = np.where(even[None, :, None], mo[0:M, jr, :], mo[M:, jr, :])
        logsT = np.where(even[None, :, None], mo[M:, jr, :], mo[0:M, jr, :])
        scat_mean[:, flat, :] = meanT.transpose(2, 1, 0)
        scat_logstd[:, flat, :] = logsT.transpose(2, 1, 0)

    inc_add = inc_addT.transpose(1, 0, 2).copy()
    inc_gain = inc_gainT.transpose(1, 0, 2).copy()
    scat_mean = scat_mean.reshape(B, R, R, M)
    scat_logstd = scat_logstd.reshape(B, R, R, M)
    return inc_add, inc_gain, scat_mean, scat_logstd, scat_mean.copy()


# revision 3
# speedup vs baseline: 1.3710x; 1.3710x over previous
"""Trainium2 Bass kernel for the BatchedEdges GNN message-passing module.

Strategy (edge-parallel over 8 NeuronCores):
  - The E=4032 directed edges are dealt to the 8 cores balanced by target
    region, grouped into runs of GS=8 slots sharing one target (padded with
    zero-weight dummy slots).
  - Per slot (edge) on device:
      GEMM1: psum[128,16] = W1[e]-slab ([S=128, 2M=128], stationary) gives
             [meanT;logstdT] stacked on partitions (order swapped for odd
             slots so meanT of odd slots lands on partitions 64-127).
  - Per group: one DVE add applies the (host-expanded) bias to the whole
    [128, 8, 16] GEMM1 block -> msgt staging; two strided DVE adds build the
    per-pair "stacked means" tile msg2[128, 4, 16] (even slot's meanT on
    partitions 0-63, odd slot's on 64-127).
  - GEMM2 (per pair of slots, f32r for full-rate fp32): stationary
    msg2[:,q,:] [128,16]; moving the fused+interleaved [addW^T | gainW^T]
    slab [128, 256]; K=128 contracts BOTH slots at once; accumulates over
    the group's 4 pairs into one PSUM tile [16, 256] = [add | gain] sums
    for the group's target.
  - Group epilogue: one scalar-engine copy of the accumulator to SBUF, DMA
    out; the host scatter-adds groups->targets and across cores.
  - All weight slabs are pre-transposed/interleaved on the host so every DMA
    lands on 128 SBUF partitions with >=512B contiguous per partition row.

The host does gather (source rows per edge), all layout packing, the final
scatter into the dense [B,R,R,M] outputs, and the 8-way partial reduction of
the [B,R,L] incoming-message sums.
"""

import numpy as np

B, R, S, M, L = 16, 64, 128, 64, 128
NC = 8
GS = 8  # slots per PSUM accumulation group (one target per group)

_F32 = np.float32
_prog_cache = {}


def _split_excess_waits(nc, cap=1):
    """This container's walrus rejects >1 semaphore wait per instruction;
    move excess waits onto preceding same-engine no-ops."""
    import concourse.mybir as mybir

    for fn in nc.m.functions:
        for blk in fn.blocks:
            insts = list(blk.instructions)
            out = []
            changed = False
            for inst in insts:
                si = getattr(inst, "sync_info", None)
                if si is not None and si.on_wait is not None and len(si.on_wait) > cap:
                    waits = list(si.on_wait)
                    excess, keep = waits[:-cap], waits[-cap:]
                    for ci in range(0, len(excess), cap):
                        out.append(
                            mybir.InstNoOp(
                                name=f"{inst.name}-wsplit{ci}",
                                engine=inst.engine,
                                ins=[],
                                outs=[],
                                sync_info=mybir.SyncInfo(
                                    on_wait=excess[ci : ci + cap], on_update=[]
                                ),
                            )
                        )
                    si.on_wait = keep
                    changed = True
                out.append(inst)
            if changed:
                blk.instructions = out


def _build_program(ng, bufs=4):
    import concourse.bass as bass
    import concourse.tile as tile
    from concourse import mybir

    f32 = mybir.dt.float32
    f32r = mybir.dt.float32r
    add_op = mybir.AluOpType.add
    act_copy = mybir.ActivationFunctionType.Copy
    nslot = ng * GS
    nc = bass.Bass()
    w1 = nc.dram_tensor("w1", [S, nslot, 2 * M], f32, kind="ExternalInput")
    ag = nc.dram_tensor("ag", [2 * M, nslot // 2, 2 * L], f32r, kind="ExternalInput")
    sg = nc.dram_tensor("sg", [S, nslot, B], f32, kind="ExternalInput")
    bx = nc.dram_tensor("bx", [2 * M, nslot, B], f32, kind="ExternalInput")
    mo = nc.dram_tensor("mo", [2 * M, nslot, B], f32, kind="ExternalOutput")
    io = nc.dram_tensor("io", [B, ng, 2 * L], f32, kind="ExternalOutput")

    with tile.TileContext(nc) as tc:
        with (
            tc.tile_pool(name="wp", bufs=bufs) as wp,
            tc.tile_pool(name="sp", bufs=bufs) as sp,
            tc.tile_pool(name="op", bufs=3) as op,
            tc.tile_pool(name="ps1p", bufs=3, space="PSUM") as ps1p,
            tc.tile_pool(name="ps2p", bufs=3, space="PSUM") as ps2p,
        ):
            for g in range(ng):
                gsl = slice(g * GS, (g + 1) * GS)
                hsl = slice(g * (GS // 2), (g + 1) * (GS // 2))
                w1t = wp.tile([S, GS, 2 * M], f32, tag="w1t")
                agt = wp.tile([2 * M, GS // 2, 2 * L], f32r, tag="agt")
                sgt = sp.tile([S, GS, B], f32, tag="sgt")
                bxt = sp.tile([2 * M, GS, B], f32, tag="bxt")
                ott = op.tile([2 * M, GS, B], f32, tag="ott")
                msg2 = op.tile([S, GS // 2, B], f32r, tag="msg2")
                incst = op.tile([B, 2 * L], f32, tag="incst")
                ps1 = ps1p.tile([S, GS, B], f32, tag="ps1")
                ps2 = ps2p.tile([B, 2 * L], f32, tag="ps2")
                nc.sync.dma_start(w1t[:], w1[:, gsl, :])
                nc.sync.dma_start(agt[:], ag[:, hsl, :])
                nc.sync.dma_start(sgt[:], sg[:, gsl, :])
                nc.sync.dma_start(bxt[:], bx[:, gsl, :])
                for i in range(GS):
                    nc.tensor.matmul(
                        ps1[:, i, :], w1t[:, i, :], sgt[:, i, :], start=True, stop=True
                    )
                nc.vector.tensor_tensor(ott[:], ps1[:], bxt[:], add_op)
                nc.vector.tensor_tensor(
                    msg2[0:M, :, :], ps1[0:M, 0::2, :], bxt[0:M, 0::2, :], add_op
                )
                nc.vector.tensor_tensor(
                    msg2[M:, :, :], ps1[M:, 1::2, :], bxt[M:, 1::2, :], add_op
                )
                for q in range(GS // 2):
                    nc.tensor.matmul(
                        ps2[:],
                        msg2[:, q, :],
                        agt[:, q, :],
                        start=(q == 0),
                        stop=(q == GS // 2 - 1),
                    )
                nc.scalar.activation(incst[:], ps2[:], act_copy)
                nc.sync.dma_start(mo[:, gsl, :], ott[:])
                nc.sync.dma_start(io[:, g, :], incst[:])

    _split_excess_waits(nc)
    return nc


def _get_program(ng):
    if ng not in _prog_cache:
        _prog_cache[ng] = _build_program(ng)
    return _prog_cache[ng]


def _plan(src_idx, tgt_idx):
    """Deal edges to cores balanced per target; group into GS-slot runs."""
    order = np.argsort(tgt_idx, kind="stable")
    sorted_tgt = tgt_idx[order]
    bounds = np.searchsorted(sorted_tgt, np.arange(R + 1))
    core_slots = [[] for _ in range(NC)]
    core_gtgt = [[] for _ in range(NC)]
    for t in range(R):
        eds = order[bounds[t] : bounds[t + 1]]
        if len(eds) == 0:
            continue
        for k in range(NC):
            chunk = eds[(k + t) % NC :: NC]
            if len(chunk) == 0:
                continue
            npad = (-len(chunk)) % GS
            core_slots[k].extend(chunk.tolist())
            core_slots[k].extend([-1] * npad)
            core_gtgt[k].extend([t] * ((len(chunk) + npad) // GS))
    ng = max(len(gt) for gt in core_gtgt)
    for k in range(NC):
        pad_groups = ng - len(core_gtgt[k])
        core_slots[k].extend([-1] * (pad_groups * GS))
        core_gtgt[k].extend([-1] * pad_groups)
    slots = [np.asarray(s, np.int64) for s in core_slots]
    gtgt = [np.asarray(t, np.int64) for t in core_gtgt]
    return ng, slots, gtgt


def kernel(**inputs):
    source = np.ascontiguousarray(np.asarray(inputs["source"], _F32))
    mean_w = np.asarray(inputs["mean_w"], _F32)
    mean_b = np.asarray(inputs["mean_b"], _F32)
    logstd_w = np.asarray(inputs["logstd_w"], _F32)
    logstd_b = np.asarray(inputs["logstd_b"], _F32)
    add_w = np.asarray(inputs["add_w"], _F32)
    gain_w = np.asarray(inputs["gain_w"], _F32)
    src_idx = np.asarray(inputs["src_idx"]).astype(np.int64).ravel()
    tgt_idx = np.asarray(inputs["tgt_idx"]).astype(np.int64).ravel()

    ng, slots, gtgt = _plan(src_idx, tgt_idx)
    nslot = ng * GS
    srcT = np.ascontiguousarray(source.transpose(2, 1, 0))  # [S, R, B]

    in_maps = []
    for k in range(NC):
        sl = slots[k]
        real = sl >= 0
        jr = np.nonzero(real)[0]
        par = jr % 2
        je, jo = jr[par == 0], jr[par == 1]
        ide, ido = sl[je], sl[jo]

        w1 = np.zeros((S, nslot, 2 * M), _F32)
        w1[:, je, 0:M] = mean_w[ide].transpose(2, 0, 1)
        w1[:, je, M:] = logstd_w[ide].transpose(2, 0, 1)
        w1[:, jo, 0:M] = logstd_w[ido].transpose(2, 0, 1)
        w1[:, jo, M:] = mean_w[ido].transpose(2, 0, 1)

        ag = np.zeros((2 * M, nslot // 2, 2 * L), _F32)
        ag[0:M, je // 2, 0:L] = add_w[ide].transpose(2, 0, 1)
        ag[0:M, je // 2, L:] = gain_w[ide].transpose(2, 0, 1)
        ag[M:, jo // 2, 0:L] = add_w[ido].transpose(2, 0, 1)
        ag[M:, jo // 2, L:] = gain_w[ido].transpose(2, 0, 1)

        sg = np.zeros((S, nslot, B), _F32)
        sg[:, jr, :] = srcT[:, src_idx[sl[jr]], :]

        bi = np.zeros((2 * M, nslot), _F32)
        bi[0:M, je] = mean_b[ide].T
        bi[M:, je] = logstd_b[ide].T
        bi[0:M, jo] = logstd_b[ido].T
        bi[M:, jo] = mean_b[ido].T
        bx = np.ascontiguousarray(
            np.broadcast_to(bi[:, :, None], (2 * M, nslot, B))
        )

        in_maps.append({"w1": w1, "ag": ag, "sg": sg, "bx": bx})

    from concourse.bass_utils import run_bass_kernel_spmd

    nc = _get_program(ng)
    res = run_bass_kernel_spmd(nc, in_maps, list(range(NC)))

    inc_addT = np.zeros((R, B, L), _F32)
    inc_gainT = np.zeros((R, B, L), _F32)
    scat_mean = np.zeros((B, R * R, M), _F32)
    scat_logstd = np.zeros((B, R * R, M), _F32)
    for k in range(NC):
        out = res.results[k]
        mo = out["mo"]  # [2M, nslot, B]
        io = out["io"]  # [B, ng, 2L]
        gt = gtgt[k]
        valid = gt >= 0
        gsum = io[:, valid, :]  # [B, ngv, 2L]
        tv = gt[valid]
        np.add.at(inc_addT, tv, gsum[:, :, 0:L].transpose(1, 0, 2))
        np.add.at(inc_gainT, tv, gsum[:, :, L:].transpose(1, 0, 2))

        sl = slots[k]
        jr = np.nonzero(sl >= 0)[0]
        eids = sl[jr]
        flat = src_idx[eids] * R + tgt_idx[eids]
        even = jr % 2 == 0
        meanT = np.where(even[None, :, None], mo[0:M, jr, :], mo[M:, jr, :])
        logsT = np.where(even[None, :, None], mo[M:, jr, :], mo[0:M, jr, :])
        scat_mean[:, flat, :] = meanT.transpose(2, 1, 0)
        scat_logstd[:, flat, :] = logsT.transpose(2, 1, 0)

    inc_add = inc_addT.transpose(1, 0, 2).copy()
    inc_gain = inc_gainT.transpose(1, 0, 2).copy()
    scat_mean = scat_mean.reshape(B, R, R, M)
    scat_logstd = scat_logstd.reshape(B, R, R, M)
    return inc_add, inc_gain, scat_mean, scat_logstd, scat_mean.copy()


# revision 4
# speedup vs baseline: 1.7614x; 1.2848x over previous
"""Trainium2 Bass kernel for the BatchedEdges GNN message-passing module.

Strategy (edge-parallel over 8 NeuronCores):
  - The E=4032 directed edges are dealt to the 8 cores balanced by target
    region, grouped into runs of GS=8 slots sharing one target (padded with
    zero-weight dummy slots).
  - Per slot (edge) on device:
      GEMM1: psum[128,16] = W1[e]-slab ([S=128, 2M=128], stationary) gives
             [meanT;logstdT] stacked on partitions (order swapped for odd
             slots so meanT of odd slots lands on partitions 64-127).
  - Per group: one DVE add applies the (host-expanded) bias to the whole
    [128, 8, 16] GEMM1 block -> msgt staging; two strided DVE adds build the
    per-pair "stacked means" tile msg2[128, 4, 16] (even slot's meanT on
    partitions 0-63, odd slot's on 64-127).
  - GEMM2 (per pair of slots, f32r for full-rate fp32): stationary
    msg2[:,q,:] [128,16]; moving the fused+interleaved [addW^T | gainW^T]
    slab [128, 256]; K=128 contracts BOTH slots at once; accumulates over
    the group's 4 pairs into one PSUM tile [16, 256] = [add | gain] sums
    for the group's target.
  - Group epilogue: one scalar-engine copy of the accumulator to SBUF, DMA
    out; the host scatter-adds groups->targets and across cores.
  - All weight slabs are pre-transposed/interleaved on the host so every DMA
    lands on 128 SBUF partitions with >=512B contiguous per partition row.

The host does gather (source rows per edge), all layout packing, the final
scatter into the dense [B,R,R,M] outputs, and the 8-way partial reduction of
the [B,R,L] incoming-message sums.
"""

import numpy as np

B, R, S, M, L = 16, 64, 128, 64, 128
NC = 8
GS = 8  # slots per PSUM accumulation group (one target per group)

_F32 = np.float32
_prog_cache = {}


def _split_excess_waits(nc, cap=1):
    """This container's walrus rejects >1 semaphore wait per instruction;
    move excess waits onto preceding same-engine no-ops."""
    import concourse.mybir as mybir

    for fn in nc.m.functions:
        for blk in fn.blocks:
            insts = list(blk.instructions)
            out = []
            changed = False
            for inst in insts:
                si = getattr(inst, "sync_info", None)
                if si is not None and si.on_wait is not None and len(si.on_wait) > cap:
                    waits = list(si.on_wait)
                    excess, keep = waits[:-cap], waits[-cap:]
                    for ci in range(0, len(excess), cap):
                        out.append(
                            mybir.InstNoOp(
                                name=f"{inst.name}-wsplit{ci}",
                                engine=inst.engine,
                                ins=[],
                                outs=[],
                                sync_info=mybir.SyncInfo(
                                    on_wait=excess[ci : ci + cap], on_update=[]
                                ),
                            )
                        )
                    si.on_wait = keep
                    changed = True
                out.append(inst)
            if changed:
                blk.instructions = out


def _build_program(ng, bufs=4):
    import concourse.bass as bass
    import concourse.tile as tile
    from concourse import mybir

    f32 = mybir.dt.float32
    f32r = mybir.dt.float32r
    add_op = mybir.AluOpType.add
    act_copy = mybir.ActivationFunctionType.Copy
    nslot = ng * GS
    nc = bass.Bass()
    w1 = nc.dram_tensor("w1", [S, nslot, 2 * M], f32r, kind="ExternalInput")
    ag = nc.dram_tensor("ag", [2 * M, nslot // 2, 2 * L], f32r, kind="ExternalInput")
    sg = nc.dram_tensor("sg", [S, nslot, B], f32r, kind="ExternalInput")
    bx = nc.dram_tensor("bx", [2 * M, nslot, B], f32, kind="ExternalInput")
    mo = nc.dram_tensor("mo", [2 * M, nslot, B], f32, kind="ExternalOutput")
    io = nc.dram_tensor("io", [B, ng, 2 * L], f32, kind="ExternalOutput")

    with tile.TileContext(nc) as tc:
        with (
            tc.tile_pool(name="wp", bufs=bufs) as wp,
            tc.tile_pool(name="sp", bufs=bufs) as sp,
            tc.tile_pool(name="op", bufs=3) as op,
            tc.tile_pool(name="ps1p", bufs=3, space="PSUM") as ps1p,
            tc.tile_pool(name="ps2p", bufs=3, space="PSUM") as ps2p,
        ):
            for g in range(ng):
                gsl = slice(g * GS, (g + 1) * GS)
                hsl = slice(g * (GS // 2), (g + 1) * (GS // 2))
                w1t = wp.tile([S, GS, 2 * M], f32r, tag="w1t")
                agt = wp.tile([2 * M, GS // 2, 2 * L], f32r, tag="agt")
                sgt = sp.tile([S, GS, B], f32r, tag="sgt")
                bxt = sp.tile([2 * M, GS, B], f32, tag="bxt")
                ott = op.tile([2 * M, GS, B], f32, tag="ott")
                msg2 = op.tile([S, GS // 2, B], f32r, tag="msg2")
                incst = op.tile([B, 2 * L], f32, tag="incst")
                ps1 = ps1p.tile([S, GS, B], f32, tag="ps1")
                ps2 = ps2p.tile([B, 2 * L], f32, tag="ps2")
                nc.sync.dma_start(w1t[:], w1[:, gsl, :])
                nc.sync.dma_start(agt[:], ag[:, hsl, :])
                nc.sync.dma_start(sgt[:], sg[:, gsl, :])
                nc.sync.dma_start(bxt[:], bx[:, gsl, :])
                for i in range(GS):
                    nc.tensor.matmul(
                        ps1[:, i, :], w1t[:, i, :], sgt[:, i, :], start=True, stop=True
                    )
                nc.vector.tensor_tensor(ott[:], ps1[:], bxt[:], add_op)
                nc.vector.tensor_tensor(
                    msg2[0:M, :, :], ps1[0:M, 0::2, :], bxt[0:M, 0::2, :], add_op
                )
                nc.vector.tensor_tensor(
                    msg2[M:, :, :], ps1[M:, 1::2, :], bxt[M:, 1::2, :], add_op
                )
                for q in range(GS // 2):
                    nc.tensor.matmul(
                        ps2[:],
                        msg2[:, q, :],
                        agt[:, q, :],
                        start=(q == 0),
                        stop=(q == GS // 2 - 1),
                    )
                nc.scalar.activation(incst[:], ps2[:], act_copy)
                nc.sync.dma_start(mo[:, gsl, :], ott[:])
                nc.sync.dma_start(io[:, g, :], incst[:])

    _split_excess_waits(nc)
    return nc


def _get_program(ng):
    if ng not in _prog_cache:
        _prog_cache[ng] = _build_program(ng)
    return _prog_cache[ng]


def _plan(src_idx, tgt_idx):
    """Deal edges to cores balanced per target; group into GS-slot runs."""
    order = np.argsort(tgt_idx, kind="stable")
    sorted_tgt = tgt_idx[order]
    bounds = np.searchsorted(sorted_tgt, np.arange(R + 1))
    core_slots = [[] for _ in range(NC)]
    core_gtgt = [[] for _ in range(NC)]
    for t in range(R):
        eds = order[bounds[t] : bounds[t + 1]]
        if len(eds) == 0:
            continue
        for k in range(NC):
            chunk = eds[(k + t) % NC :: NC]
            if len(chunk) == 0:
                continue
            npad = (-len(chunk)) % GS
            core_slots[k].extend(chunk.tolist())
            core_slots[k].extend([-1] * npad)
            core_gtgt[k].extend([t] * ((len(chunk) + npad) // GS))
    ng = max(len(gt) for gt in core_gtgt)
    for k in range(NC):
        pad_groups = ng - len(core_gtgt[k])
        core_slots[k].extend([-1] * (pad_groups * GS))
        core_gtgt[k].extend([-1] * pad_groups)
    slots = [np.asarray(s, np.int64) for s in core_slots]
    gtgt = [np.asarray(t, np.int64) for t in core_gtgt]
    return ng, slots, gtgt


def kernel(**inputs):
    source = np.ascontiguousarray(np.asarray(inputs["source"], _F32))
    mean_w = np.asarray(inputs["mean_w"], _F32)
    mean_b = np.asarray(inputs["mean_b"], _F32)
    logstd_w = np.asarray(inputs["logstd_w"], _F32)
    logstd_b = np.asarray(inputs["logstd_b"], _F32)
    add_w = np.asarray(inputs["add_w"], _F32)
    gain_w = np.asarray(inputs["gain_w"], _F32)
    src_idx = np.asarray(inputs["src_idx"]).astype(np.int64).ravel()
    tgt_idx = np.asarray(inputs["tgt_idx"]).astype(np.int64).ravel()

    ng, slots, gtgt = _plan(src_idx, tgt_idx)
    nslot = ng * GS
    srcT = np.ascontiguousarray(source.transpose(2, 1, 0))  # [S, R, B]

    in_maps = []
    for k in range(NC):
        sl = slots[k]
        real = sl >= 0
        jr = np.nonzero(real)[0]
        par = jr % 2
        je, jo = jr[par == 0], jr[par == 1]
        ide, ido = sl[je], sl[jo]

        w1 = np.zeros((S, nslot, 2 * M), _F32)
        w1[:, je, 0:M] = mean_w[ide].transpose(2, 0, 1)
        w1[:, je, M:] = logstd_w[ide].transpose(2, 0, 1)
        w1[:, jo, 0:M] = logstd_w[ido].transpose(2, 0, 1)
        w1[:, jo, M:] = mean_w[ido].transpose(2, 0, 1)

        ag = np.zeros((2 * M, nslot // 2, 2 * L), _F32)
        ag[0:M, je // 2, 0:L] = add_w[ide].transpose(2, 0, 1)
        ag[0:M, je // 2, L:] = gain_w[ide].transpose(2, 0, 1)
        ag[M:, jo // 2, 0:L] = add_w[ido].transpose(2, 0, 1)
        ag[M:, jo // 2, L:] = gain_w[ido].transpose(2, 0, 1)

        sg = np.zeros((S, nslot, B), _F32)
        sg[:, jr, :] = srcT[:, src_idx[sl[jr]], :]

        bi = np.zeros((2 * M, nslot), _F32)
        bi[0:M, je] = mean_b[ide].T
        bi[M:, je] = logstd_b[ide].T
        bi[0:M, jo] = logstd_b[ido].T
        bi[M:, jo] = mean_b[ido].T
        bx = np.ascontiguousarray(
            np.broadcast_to(bi[:, :, None], (2 * M, nslot, B))
        )

        in_maps.append({"w1": w1, "ag": ag, "sg": sg, "bx": bx})

    from concourse.bass_utils import run_bass_kernel_spmd

    nc = _get_program(ng)
    res = run_bass_kernel_spmd(nc, in_maps, list(range(NC)))

    inc_addT = np.zeros((R, B, L), _F32)
    inc_gainT = np.zeros((R, B, L), _F32)
    scat_mean = np.zeros((B, R * R, M), _F32)
    scat_logstd = np.zeros((B, R * R, M), _F32)
    for k in range(NC):
        out = res.results[k]
        mo = out["mo"]  # [2M, nslot, B]
        io = out["io"]  # [B, ng, 2L]
        gt = gtgt[k]
        valid = gt >= 0
        gsum = io[:, valid, :]  # [B, ngv, 2L]
        tv = gt[valid]
        np.add.at(inc_addT, tv, gsum[:, :, 0:L].transpose(1, 0, 2))
        np.add.at(inc_gainT, tv, gsum[:, :, L:].transpose(1, 0, 2))

        sl = slots[k]
        jr = np.nonzero(sl >= 0)[0]
        eids = sl[jr]
        flat = src_idx[eids] * R + tgt_idx[eids]
        even = jr % 2 == 0
        meanT = np.where(even[None, :, None], mo[0:M, jr, :], mo[M:, jr, :])
        logsT = np.where(even[None, :, None], mo[M:, jr, :], mo[0:M, jr, :])
        scat_mean[:, flat, :] = meanT.transpose(2, 1, 0)
        scat_logstd[:, flat, :] = logsT.transpose(2, 1, 0)

    inc_add = inc_addT.transpose(1, 0, 2).copy()
    inc_gain = inc_gainT.transpose(1, 0, 2).copy()
    scat_mean = scat_mean.reshape(B, R, R, M)
    scat_logstd = scat_logstd.reshape(B, R, R, M)
    return inc_add, inc_gain, scat_mean, scat_logstd, scat_mean.copy()


# revision 6
# speedup vs baseline: 2.7779x; 1.5771x over previous
"""Trainium2 Bass kernel for the BatchedEdges GNN message-passing module.

Strategy (edge-parallel over 8 NeuronCores):
  - The E=4032 directed edges are dealt to the 8 cores balanced by target
    region, grouped into runs of GS=8 slots sharing one target (padded with
    zero-weight dummy slots).
  - Per slot (edge) on device:
      GEMM1: psum[128,16] = W1[e]-slab ([S=128, 2M=128], stationary) gives
             [meanT;logstdT] stacked on partitions (order swapped for odd
             slots so meanT of odd slots lands on partitions 64-127).
  - Per group: one DVE add applies the (host-expanded) bias to the whole
    [128, 8, 16] GEMM1 block -> msgt staging; two strided DVE adds build the
    per-pair "stacked means" tile msg2[128, 4, 16] (even slot's meanT on
    partitions 0-63, odd slot's on 64-127).
  - GEMM2 (per pair of slots, f32r for full-rate fp32): stationary
    msg2[:,q,:] [128,16]; moving the fused+interleaved [addW^T | gainW^T]
    slab [128, 256]; K=128 contracts BOTH slots at once; accumulates over
    the group's 4 pairs into one PSUM tile [16, 256] = [add | gain] sums
    for the group's target.
  - Group epilogue: one scalar-engine copy of the accumulator to SBUF, DMA
    out; the host scatter-adds groups->targets and across cores.
  - All weight slabs are pre-transposed/interleaved on the host so every DMA
    lands on 128 SBUF partitions with >=512B contiguous per partition row.

The host does gather (source rows per edge), all layout packing, the final
scatter into the dense [B,R,R,M] outputs, and the 8-way partial reduction of
the [B,R,L] incoming-message sums.
"""

import numpy as np

B, R, S, M, L = 16, 64, 128, 64, 128
NC = 8
GS = 8  # slots per PSUM accumulation group (one target per group)

_F32 = np.float32
_prog_cache = {}


def _split_excess_waits(nc, cap=1):
    """This container's walrus rejects >1 semaphore wait per instruction;
    move excess waits onto preceding same-engine no-ops."""
    import concourse.mybir as mybir

    for fn in nc.m.functions:
        for blk in fn.blocks:
            insts = list(blk.instructions)
            out = []
            changed = False
            for inst in insts:
                si = getattr(inst, "sync_info", None)
                if si is not None and si.on_wait is not None and len(si.on_wait) > cap:
                    waits = list(si.on_wait)
                    excess, keep = waits[:-cap], waits[-cap:]
                    for ci in range(0, len(excess), cap):
                        out.append(
                            mybir.InstNoOp(
                                name=f"{inst.name}-wsplit{ci}",
                                engine=inst.engine,
                                ins=[],
                                outs=[],
                                sync_info=mybir.SyncInfo(
                                    on_wait=excess[ci : ci + cap], on_update=[]
                                ),
                            )
                        )
                    si.on_wait = keep
                    changed = True
                out.append(inst)
            if changed:
                blk.instructions = out


def _build_program(ng, bufs=8):
    import concourse.bass as bass
    import concourse.tile as tile
    from concourse import mybir

    f32 = mybir.dt.float32
    f32r = mybir.dt.float32r
    add_op = mybir.AluOpType.add
    act_copy = mybir.ActivationFunctionType.Copy
    nslot = ng * GS
    nc = bass.Bass()
    w1 = nc.dram_tensor("w1", [S, nslot, 2 * M], f32r, kind="ExternalInput")
    ag = nc.dram_tensor("ag", [2 * M, nslot // 2, 2 * L], f32r, kind="ExternalInput")
    sg = nc.dram_tensor("sg", [S, nslot, B], f32r, kind="ExternalInput")
    bx = nc.dram_tensor("bx", [2 * M, nslot, B], f32, kind="ExternalInput")
    mo = nc.dram_tensor("mo", [2 * M, nslot, B], f32, kind="ExternalOutput")
    io = nc.dram_tensor("io", [B, ng, 2 * L], f32, kind="ExternalOutput")

    with tile.TileContext(nc) as tc:
        with (
            tc.tile_pool(name="wp", bufs=bufs) as wp,
            tc.tile_pool(name="sp", bufs=bufs) as sp,
            tc.tile_pool(name="op", bufs=3) as op,
            tc.tile_pool(name="ps1p", bufs=3, space="PSUM") as ps1p,
            tc.tile_pool(name="ps2p", bufs=3, space="PSUM") as ps2p,
        ):

            def emit_gemm1(g):
                gsl = slice(g * GS, (g + 1) * GS)
                hsl = slice(g * (GS // 2), (g + 1) * (GS // 2))
                w1t = wp.tile([S, GS, 2 * M], f32r, tag="w1t")
                agt = wp.tile([2 * M, GS // 2, 2 * L], f32r, tag="agt")
                sgt = sp.tile([S, GS, B], f32r, tag="sgt")
                bxt = sp.tile([2 * M, GS, B], f32, tag="bxt")
                ps1 = ps1p.tile([S, GS, B], f32, tag="ps1")
                nc.gpsimd.dma_start(w1t[:], w1[:, gsl, :])
                nc.gpsimd.dma_start(agt[:], ag[:, hsl, :])
                nc.scalar.dma_start(sgt[:], sg[:, gsl, :])
                nc.scalar.dma_start(bxt[:], bx[:, gsl, :])
                for i in range(GS):
                    nc.tensor.matmul(
                        ps1[:, i, :], w1t[:, i, :], sgt[:, i, :], start=True, stop=True
                    )
                return g, agt, bxt, ps1

            def emit_epilogue(state):
                g, agt, bxt, ps1 = state
                gsl = slice(g * GS, (g + 1) * GS)
                ott = op.tile([2 * M, GS, B], f32, tag="ott")
                msg2 = op.tile([S, GS // 2, B], f32r, tag="msg2")
                incst = op.tile([B, 2 * L], f32, tag="incst")
                ps2 = ps2p.tile([B, 2 * L], f32, tag="ps2")
                nc.vector.tensor_tensor(ott[:], ps1[:], bxt[:], add_op)
                nc.vector.tensor_tensor(
                    msg2[0:M, :, :], ps1[0:M, 0::2, :], bxt[0:M, 0::2, :], add_op
                )
                nc.vector.tensor_tensor(
                    msg2[M:, :, :], ps1[M:, 1::2, :], bxt[M:, 1::2, :], add_op
                )
                for q in range(GS // 2):
                    nc.tensor.matmul(
                        ps2[:],
                        msg2[:, q, :],
                        agt[:, q, :],
                        start=(q == 0),
                        stop=(q == GS // 2 - 1),
                    )
                nc.scalar.activation(incst[:], ps2[:], act_copy)
                nc.sync.dma_start(mo[:, gsl, :], ott[:])
                nc.sync.dma_start(io[:, g, :], incst[:])

            prev = None
            for g in range(ng):
                cur = emit_gemm1(g)
                if prev is not None:
                    emit_epilogue(prev)
                prev = cur
            emit_epilogue(prev)

    _split_excess_waits(nc)
    return nc


def _get_program(ng):
    if ng not in _prog_cache:
        _prog_cache[ng] = _build_program(ng)
    return _prog_cache[ng]


def _plan(src_idx, tgt_idx):
    """Deal edges to cores balanced per target; group into GS-slot runs."""
    order = np.argsort(tgt_idx, kind="stable")
    sorted_tgt = tgt_idx[order]
    bounds = np.searchsorted(sorted_tgt, np.arange(R + 1))
    core_slots = [[] for _ in range(NC)]
    core_gtgt = [[] for _ in range(NC)]
    for t in range(R):
        eds = order[bounds[t] : bounds[t + 1]]
        if len(eds) == 0:
            continue
        for k in range(NC):
            chunk = eds[(k + t) % NC :: NC]
            if len(chunk) == 0:
                continue
            npad = (-len(chunk)) % GS
            core_slots[k].extend(chunk.tolist())
            core_slots[k].extend([-1] * npad)
            core_gtgt[k].extend([t] * ((len(chunk) + npad) // GS))
    ng = max(len(gt) for gt in core_gtgt)
    for k in range(NC):
        pad_groups = ng - len(core_gtgt[k])
        core_slots[k].extend([-1] * (pad_groups * GS))
        core_gtgt[k].extend([-1] * pad_groups)
    slots = [np.asarray(s, np.int64) for s in core_slots]
    gtgt = [np.asarray(t, np.int64) for t in core_gtgt]
    return ng, slots, gtgt


def kernel(**inputs):
    source = np.ascontiguousarray(np.asarray(inputs["source"], _F32))
    mean_w = np.asarray(inputs["mean_w"], _F32)
    mean_b = np.asarray(inputs["mean_b"], _F32)
    logstd_w = np.asarray(inputs["logstd_w"], _F32)
    logstd_b = np.asarray(inputs["logstd_b"], _F32)
    add_w = np.asarray(inputs["add_w"], _F32)
    gain_w = np.asarray(inputs["gain_w"], _F32)
    src_idx = np.asarray(inputs["src_idx"]).astype(np.int64).ravel()
    tgt_idx = np.asarray(inputs["tgt_idx"]).astype(np.int64).ravel()

    ng, slots, gtgt = _plan(src_idx, tgt_idx)
    nslot = ng * GS
    srcT = np.ascontiguousarray(source.transpose(2, 1, 0))  # [S, R, B]

    in_maps = []
    for k in range(NC):
        sl = slots[k]
        real = sl >= 0
        jr = np.nonzero(real)[0]
        par = jr % 2
        je, jo = jr[par == 0], jr[par == 1]
        ide, ido = sl[je], sl[jo]

        w1 = np.zeros((S, nslot, 2 * M), _F32)
        w1[:, je, 0:M] = mean_w[ide].transpose(2, 0, 1)
        w1[:, je, M:] = logstd_w[ide].transpose(2, 0, 1)
        w1[:, jo, 0:M] = logstd_w[ido].transpose(2, 0, 1)
        w1[:, jo, M:] = mean_w[ido].transpose(2, 0, 1)

        ag = np.zeros((2 * M, nslot // 2, 2 * L), _F32)
        ag[0:M, je // 2, 0:L] = add_w[ide].transpose(2, 0, 1)
        ag[0:M, je // 2, L:] = gain_w[ide].transpose(2, 0, 1)
        ag[M:, jo // 2, 0:L] = add_w[ido].transpose(2, 0, 1)
        ag[M:, jo // 2, L:] = gain_w[ido].transpose(2, 0, 1)

        sg = np.zeros((S, nslot, B), _F32)
        sg[:, jr, :] = srcT[:, src_idx[sl[jr]], :]

        bi = np.zeros((2 * M, nslot), _F32)
        bi[0:M, je] = mean_b[ide].T
        bi[M:, je] = logstd_b[ide].T
        bi[0:M, jo] = logstd_b[ido].T
        bi[M:, jo] = mean_b[ido].T
        bx = np.ascontiguousarray(
            np.broadcast_to(bi[:, :, None], (2 * M, nslot, B))
        )

        in_maps.append({"w1": w1, "ag": ag, "sg": sg, "bx": bx})

    from concourse.bass_utils import run_bass_kernel_spmd

    nc = _get_program(ng)
    res = run_bass_kernel_spmd(nc, in_maps, list(range(NC)))

    inc_addT = np.zeros((R, B, L), _F32)
    inc_gainT = np.zeros((R, B, L), _F32)
    scat_mean = np.zeros((B, R * R, M), _F32)
    scat_logstd = np.zeros((B, R * R, M), _F32)
    for k in range(NC):
        out = res.results[k]
        mo = out["mo"]  # [2M, nslot, B]
        io = out["io"]  # [B, ng, 2L]
        gt = gtgt[k]
        valid = gt >= 0
        gsum = io[:, valid, :]  # [B, ngv, 2L]
        tv = gt[valid]
        np.add.at(inc_addT, tv, gsum[:, :, 0:L].transpose(1, 0, 2))
        np.add.at(inc_gainT, tv, gsum[:, :, L:].transpose(1, 0, 2))

        sl = slots[k]
        jr = np.nonzero(sl >= 0)[0]
        eids = sl[jr]
        flat = src_idx[eids] * R + tgt_idx[eids]
        even = jr % 2 == 0
        meanT = np.where(even[None, :, None], mo[0:M, jr, :], mo[M:, jr, :])
        logsT = np.where(even[None, :, None], mo[M:, jr, :], mo[0:M, jr, :])
        scat_mean[:, flat, :] = meanT.transpose(2, 1, 0)
        scat_logstd[:, flat, :] = logsT.transpose(2, 1, 0)

    inc_add = inc_addT.transpose(1, 0, 2).copy()
    inc_gain = inc_gainT.transpose(1, 0, 2).copy()
    scat_mean = scat_mean.reshape(B, R, R, M)
    scat_logstd = scat_logstd.reshape(B, R, R, M)
    return inc_add, inc_gain, scat_mean, scat_logstd, scat_mean.copy()


# revision 8
# speedup vs baseline: 3.6553x; 1.3159x over previous
"""Trainium2 Bass kernel for the BatchedEdges GNN message-passing module.

Strategy (edge-parallel over 8 NeuronCores):
  - The E=4032 directed edges are dealt to the 8 cores balanced by target
    region, grouped into runs of GS=8 slots sharing one target (padded with
    zero-weight dummy slots).
  - Per slot (edge) on device:
      GEMM1: psum[128,16] = W1[e]-slab ([S=128, 2M=128], stationary) gives
             [meanT;logstdT] stacked on partitions (order swapped for odd
             slots so meanT of odd slots lands on partitions 64-127).
  - Per group: one DVE add applies the (host-expanded) bias to the whole
    [128, 8, 16] GEMM1 block -> msgt staging; two strided DVE adds build the
    per-pair "stacked means" tile msg2[128, 4, 16] (even slot's meanT on
    partitions 0-63, odd slot's on 64-127).
  - GEMM2 (per pair of slots, f32r for full-rate fp32): stationary
    msg2[:,q,:] [128,16]; moving the fused+interleaved [addW^T | gainW^T]
    slab [128, 256]; K=128 contracts BOTH slots at once; accumulates over
    the group's 4 pairs into one PSUM tile [16, 256] = [add | gain] sums
    for the group's target.
  - Group epilogue: one scalar-engine copy of the accumulator to SBUF, DMA
    out; the host scatter-adds groups->targets and across cores.
  - All weight slabs are pre-transposed/interleaved on the host so every DMA
    lands on 128 SBUF partitions with >=512B contiguous per partition row.

The host does gather (source rows per edge), all layout packing, the final
scatter into the dense [B,R,R,M] outputs, and the 8-way partial reduction of
the [B,R,L] incoming-message sums.
"""

import numpy as np

B, R, S, M, L = 16, 64, 128, 64, 128
NC = 8
GS = 8  # slots per PSUM accumulation group (one target per group)

_F32 = np.float32
_prog_cache = {}


def _split_excess_waits(nc, cap=1):
    """This container's walrus rejects >1 semaphore wait per instruction;
    move excess waits onto preceding same-engine no-ops."""
    import concourse.mybir as mybir

    for fn in nc.m.functions:
        for blk in fn.blocks:
            insts = list(blk.instructions)
            out = []
            changed = False
            for inst in insts:
                si = getattr(inst, "sync_info", None)
                if si is not None and si.on_wait is not None and len(si.on_wait) > cap:
                    waits = list(si.on_wait)
                    excess, keep = waits[:-cap], waits[-cap:]
                    for ci in range(0, len(excess), cap):
                        out.append(
                            mybir.InstNoOp(
                                name=f"{inst.name}-wsplit{ci}",
                                engine=inst.engine,
                                ins=[],
                                outs=[],
                                sync_info=mybir.SyncInfo(
                                    on_wait=excess[ci : ci + cap], on_update=[]
                                ),
                            )
                        )
                    si.on_wait = keep
                    changed = True
                out.append(inst)
            if changed:
                blk.instructions = out


def _build_program(ng, bufs=8):
    import concourse.bass as bass
    import concourse.tile as tile
    from concourse import mybir

    f32 = mybir.dt.float32
    f32r = mybir.dt.float32r
    add_op = mybir.AluOpType.add
    act_copy = mybir.ActivationFunctionType.Copy
    nslot = ng * GS
    nc = bass.Bass()
    w1 = nc.dram_tensor("w1", [S, nslot, 2 * M], f32r, kind="ExternalInput")
    ag = nc.dram_tensor("ag", [2 * M, nslot // 2, 2 * L], f32r, kind="ExternalInput")
    sg = nc.dram_tensor("sg", [S, nslot, B], f32r, kind="ExternalInput")
    bi = nc.dram_tensor("bi", [2 * M, nslot], f32, kind="ExternalInput")
    mo = nc.dram_tensor("mo", [2 * M, nslot, B], f32, kind="ExternalOutput")
    io = nc.dram_tensor("io", [B, ng, 2 * L], f32, kind="ExternalOutput")

    with tile.TileContext(nc) as tc:
        with (
            tc.tile_pool(name="wp", bufs=bufs) as wp,
            tc.tile_pool(name="sp", bufs=bufs) as sp,
            tc.tile_pool(name="op", bufs=12) as op,
            tc.tile_pool(name="cp", bufs=1) as cp,
            tc.tile_pool(name="ps1p", bufs=4, space="PSUM") as ps1p,
            tc.tile_pool(name="ps2p", bufs=4, space="PSUM") as ps2p,
        ):
            bit = cp.tile([2 * M, nslot], f32, tag="bit")
            nc.sync.dma_start(bit[:], bi[:])

            def emit_gemm1(g):
                gsl = slice(g * GS, (g + 1) * GS)
                hsl = slice(g * (GS // 2), (g + 1) * (GS // 2))
                w1t = wp.tile([S, GS, 2 * M], f32r, tag="w1t")
                agt = wp.tile([2 * M, GS // 2, 2 * L], f32r, tag="agt")
                sgt = sp.tile([S, GS, B], f32r, tag="sgt")
                ps1 = ps1p.tile([S, GS, B], f32, tag="ps1")
                nc.gpsimd.dma_start(w1t[:], w1[:, gsl, :])
                nc.gpsimd.dma_start(agt[:], ag[:, hsl, :])
                nc.scalar.dma_start(sgt[:], sg[:, gsl, :])
                for i in range(GS):
                    nc.tensor.matmul(
                        ps1[:, i, :], w1t[:, i, :], sgt[:, i, :], start=True, stop=True
                    )
                return g, agt, ps1

            def emit_epilogue(state):
                g, agt, ps1 = state
                gsl = slice(g * GS, (g + 1) * GS)
                ott = op.tile([2 * M, GS, B], f32, tag="ott")
                msg2 = op.tile([S, GS // 2, B], f32r, tag="msg2")
                incst = op.tile([B, 2 * L], f32, tag="incst")
                ps2 = ps2p.tile([B, 2 * L], f32, tag="ps2")
                bslab = bit[:, gsl]
                nc.vector.tensor_tensor(
                    ott[:], ps1[:], bslab.broadcast_to([2 * M, GS, B]), add_op
                )
                nc.vector.tensor_tensor(
                    msg2[0:M, :, :],
                    ps1[0:M, 0::2, :],
                    bslab[0:M, 0::2].broadcast_to([M, GS // 2, B]),
                    add_op,
                )
                nc.vector.tensor_tensor(
                    msg2[M:, :, :],
                    ps1[M:, 1::2, :],
                    bslab[M:, 1::2].broadcast_to([M, GS // 2, B]),
                    add_op,
                )
                for q in range(GS // 2):
                    nc.tensor.matmul(
                        ps2[:],
                        msg2[:, q, :],
                        agt[:, q, :],
                        start=(q == 0),
                        stop=(q == GS // 2 - 1),
                    )
                nc.scalar.activation(incst[:], ps2[:], act_copy)
                nc.sync.dma_start(mo[:, gsl, :], ott[:])
                nc.sync.dma_start(io[:, g, :], incst[:])

            from collections import deque

            pending = deque()
            for g in range(ng):
                pending.append(emit_gemm1(g))
                if len(pending) > 2:
                    emit_epilogue(pending.popleft())
            while pending:
                emit_epilogue(pending.popleft())

    _split_excess_waits(nc)
    return nc


def _get_program(ng):
    if ng not in _prog_cache:
        _prog_cache[ng] = _build_program(ng)
    return _prog_cache[ng]


def _plan(src_idx, tgt_idx):
    """Deal edges to cores balanced per target; group into GS-slot runs."""
    order = np.argsort(tgt_idx, kind="stable")
    sorted_tgt = tgt_idx[order]
    bounds = np.searchsorted(sorted_tgt, np.arange(R + 1))
    core_slots = [[] for _ in range(NC)]
    core_gtgt = [[] for _ in range(NC)]
    for t in range(R):
        eds = order[bounds[t] : bounds[t + 1]]
        if len(eds) == 0:
            continue
        for k in range(NC):
            chunk = eds[(k + t) % NC :: NC]
            if len(chunk) == 0:
                continue
            npad = (-len(chunk)) % GS
            core_slots[k].extend(chunk.tolist())
            core_slots[k].extend([-1] * npad)
            core_gtgt[k].extend([t] * ((len(chunk) + npad) // GS))
    ng = max(len(gt) for gt in core_gtgt)
    for k in range(NC):
        pad_groups = ng - len(core_gtgt[k])
        core_slots[k].extend([-1] * (pad_groups * GS))
        core_gtgt[k].extend([-1] * pad_groups)
    slots = [np.asarray(s, np.int64) for s in core_slots]
    gtgt = [np.asarray(t, np.int64) for t in core_gtgt]
    return ng, slots, gtgt


def kernel(**inputs):
    source = np.ascontiguousarray(np.asarray(inputs["source"], _F32))
    mean_w = np.asarray(inputs["mean_w"], _F32)
    mean_b = np.asarray(inputs["mean_b"], _F32)
    logstd_w = np.asarray(inputs["logstd_w"], _F32)
    logstd_b = np.asarray(inputs["logstd_b"], _F32)
    add_w = np.asarray(inputs["add_w"], _F32)
    gain_w = np.asarray(inputs["gain_w"], _F32)
    src_idx = np.asarray(inputs["src_idx"]).astype(np.int64).ravel()
    tgt_idx = np.asarray(inputs["tgt_idx"]).astype(np.int64).ravel()

    ng, slots, gtgt = _plan(src_idx, tgt_idx)
    nslot = ng * GS
    srcT = np.ascontiguousarray(source.transpose(2, 1, 0))  # [S, R, B]

    in_maps = []
    for k in range(NC):
        sl = slots[k]
        real = sl >= 0
        jr = np.nonzero(real)[0]
        par = jr % 2
        je, jo = jr[par == 0], jr[par == 1]
        ide, ido = sl[je], sl[jo]

        w1 = np.zeros((S, nslot, 2 * M), _F32)
        w1[:, je, 0:M] = mean_w[ide].transpose(2, 0, 1)
        w1[:, je, M:] = logstd_w[ide].transpose(2, 0, 1)
        w1[:, jo, 0:M] = logstd_w[ido].transpose(2, 0, 1)
        w1[:, jo, M:] = mean_w[ido].transpose(2, 0, 1)

        ag = np.zeros((2 * M, nslot // 2, 2 * L), _F32)
        ag[0:M, je // 2, 0:L] = add_w[ide].transpose(2, 0, 1)
        ag[0:M, je // 2, L:] = gain_w[ide].transpose(2, 0, 1)
        ag[M:, jo // 2, 0:L] = add_w[ido].transpose(2, 0, 1)
        ag[M:, jo // 2, L:] = gain_w[ido].transpose(2, 0, 1)

        sg = np.zeros((S, nslot, B), _F32)
        sg[:, jr, :] = srcT[:, src_idx[sl[jr]], :]

        bi = np.zeros((2 * M, nslot), _F32)
        bi[0:M, je] = mean_b[ide].T
        bi[M:, je] = logstd_b[ide].T
        bi[0:M, jo] = logstd_b[ido].T
        bi[M:, jo] = mean_b[ido].T
        in_maps.append({"w1": w1, "ag": ag, "sg": sg, "bi": bi})

    from concourse.bass_utils import run_bass_kernel_spmd

    nc = _get_program(ng)
    res = run_bass_kernel_spmd(nc, in_maps, list(range(NC)))

    inc_addT = np.zeros((R, B, L), _F32)
    inc_gainT = np.zeros((R, B, L), _F32)
    scat_mean = np.zeros((B, R * R, M), _F32)
    scat_logstd = np.zeros((B, R * R, M), _F32)
    for k in range(NC):
        out = res.results[k]
        mo = out["mo"]  # [2M, nslot, B]
        io = out["io"]  # [B, ng, 2L]
        gt = gtgt[k]
        valid = gt >= 0
        gsum = io[:, valid, :]  # [B, ngv, 2L]
        tv = gt[valid]
        np.add.at(inc_addT, tv, gsum[:, :, 0:L].transpose(1, 0, 2))
        np.add.at(inc_gainT, tv, gsum[:, :, L:].transpose(1, 0, 2))

        sl = slots[k]
        jr = np.nonzero(sl >= 0)[0]
        eids = sl[jr]
        flat = src_idx[eids] * R + tgt_idx[eids]
        even = jr % 2 == 0
        meanT = np.where(even[None, :, None], mo[0:M, jr, :], mo[M:, jr, :])
        logsT = np.where(even[None, :, None], mo[M:, jr, :], mo[0:M, jr, :])
        scat_mean[:, flat, :] = meanT.transpose(2, 1, 0)
        scat_logstd[:, flat, :] = logsT.transpose(2, 1, 0)

    inc_add = inc_addT.transpose(1, 0, 2).copy()
    inc_gain = inc_gainT.transpose(1, 0, 2).copy()
    scat_mean = scat_mean.reshape(B, R, R, M)
    scat_logstd = scat_logstd.reshape(B, R, R, M)
    return inc_add, inc_gain, scat_mean, scat_logstd, scat_mean.copy()


# revision 9
# speedup vs baseline: 3.6684x; 1.0036x over previous
"""Trainium2 Bass kernel for the BatchedEdges GNN message-passing module.

Strategy (edge-parallel over 8 NeuronCores):
  - The E=4032 directed edges are dealt to the 8 cores balanced by target
    region, grouped into runs of GS=8 slots sharing one target (padded with
    zero-weight dummy slots).
  - Per slot (edge) on device:
      GEMM1: psum[128,16] = W1[e]-slab ([S=128, 2M=128], stationary) gives
             [meanT;logstdT] stacked on partitions (order swapped for odd
             slots so meanT of odd slots lands on partitions 64-127).
  - Per group: one DVE add applies the (host-expanded) bias to the whole
    [128, 8, 16] GEMM1 block -> msgt staging; two strided DVE adds build the
    per-pair "stacked means" tile msg2[128, 4, 16] (even slot's meanT on
    partitions 0-63, odd slot's on 64-127).
  - GEMM2 (per pair of slots, f32r for full-rate fp32): stationary
    msg2[:,q,:] [128,16]; moving the fused+interleaved [addW^T | gainW^T]
    slab [128, 256]; K=128 contracts BOTH slots at once; accumulates over
    the group's 4 pairs into one PSUM tile [16, 256] = [add | gain] sums
    for the group's target.
  - Group epilogue: one scalar-engine copy of the accumulator to SBUF, DMA
    out; the host scatter-adds groups->targets and across cores.
  - All weight slabs are pre-transposed/interleaved on the host so every DMA
    lands on 128 SBUF partitions with >=512B contiguous per partition row.

The host does gather (source rows per edge), all layout packing, the final
scatter into the dense [B,R,R,M] outputs, and the 8-way partial reduction of
the [B,R,L] incoming-message sums.
"""

import numpy as np

B, R, S, M, L = 16, 64, 128, 64, 128
NC = 8
GS = 8  # slots per PSUM accumulation group (one target per group)

_F32 = np.float32
_prog_cache = {}


def _split_excess_waits(nc, cap=1):
    """This container's walrus rejects >1 semaphore wait per instruction;
    move excess waits onto preceding same-engine no-ops."""
    import concourse.mybir as mybir

    for fn in nc.m.functions:
        for blk in fn.blocks:
            insts = list(blk.instructions)
            out = []
            changed = False
            for inst in insts:
                si = getattr(inst, "sync_info", None)
                if si is not None and si.on_wait is not None and len(si.on_wait) > cap:
                    waits = list(si.on_wait)
                    excess, keep = waits[:-cap], waits[-cap:]
                    for ci in range(0, len(excess), cap):
                        out.append(
                            mybir.InstNoOp(
                                name=f"{inst.name}-wsplit{ci}",
                                engine=inst.engine,
                                ins=[],
                                outs=[],
                                sync_info=mybir.SyncInfo(
                                    on_wait=excess[ci : ci + cap], on_update=[]
                                ),
                            )
                        )
                    si.on_wait = keep
                    changed = True
                out.append(inst)
            if changed:
                blk.instructions = out


def _build_program(ng, bufs=8):
    import concourse.bass as bass
    import concourse.tile as tile
    from concourse import mybir

    f32 = mybir.dt.float32
    f32r = mybir.dt.float32r
    add_op = mybir.AluOpType.add
    act_copy = mybir.ActivationFunctionType.Copy
    nslot = ng * GS
    nc = bass.Bass()
    w1 = nc.dram_tensor("w1", [S, nslot, 2 * M], f32r, kind="ExternalInput")
    ag = nc.dram_tensor("ag", [2 * M, nslot // 2, 2 * L], f32r, kind="ExternalInput")
    sg = nc.dram_tensor("sg", [S, nslot, B], f32r, kind="ExternalInput")
    bi = nc.dram_tensor("bi", [2 * M, nslot], f32, kind="ExternalInput")
    mo = nc.dram_tensor("mo", [2 * M, nslot, B], f32, kind="ExternalOutput")
    io = nc.dram_tensor("io", [B, ng, 2 * L], f32, kind="ExternalOutput")

    with tile.TileContext(nc) as tc:
        with (
            tc.tile_pool(name="wp", bufs=bufs) as wp,
            tc.tile_pool(name="sp", bufs=bufs) as sp,
            tc.tile_pool(name="op", bufs=12) as op,
            tc.tile_pool(name="cp", bufs=1) as cp,
            tc.tile_pool(name="ps1p", bufs=3, space="PSUM") as ps1p,
            tc.tile_pool(name="ps2p", bufs=2, space="PSUM") as ps2p,
        ):
            bit = cp.tile([2 * M, nslot], f32, tag="bit")
            nc.sync.dma_start(bit[:], bi[:])

            def emit_gemm1(g):
                gsl = slice(g * GS, (g + 1) * GS)
                hsl = slice(g * (GS // 2), (g + 1) * (GS // 2))
                w1t = wp.tile([S, GS, 2 * M], f32r, tag="w1t")
                agt = wp.tile([2 * M, GS // 2, 2 * L], f32r, tag="agt")
                sgt = sp.tile([S, GS, B], f32r, tag="sgt")
                ps1a = ps1p.tile([S, GS // 2, B], f32, tag="ps1a")
                ps1b = ps1p.tile([S, GS // 2, B], f32, tag="ps1b")
                nc.gpsimd.dma_start(w1t[:], w1[:, gsl, :])
                nc.gpsimd.dma_start(agt[:], ag[:, hsl, :])
                nc.scalar.dma_start(sgt[:], sg[:, gsl, :])
                for i in range(GS):
                    tgt_ps = ps1a if i < GS // 2 else ps1b
                    nc.tensor.matmul(
                        tgt_ps[:, i % (GS // 2), :],
                        w1t[:, i, :],
                        sgt[:, i, :],
                        start=True,
                        stop=True,
                    )
                return g, agt, ps1a, ps1b

            def emit_epilogue(state):
                g, agt, ps1a, ps1b = state
                gsl = slice(g * GS, (g + 1) * GS)
                lsl = slice(g * GS, g * GS + GS // 2)
                hsl2 = slice(g * GS + GS // 2, (g + 1) * GS)
                ott = op.tile([2 * M, GS, B], f32, tag="ott")
                msg2 = op.tile([S, GS // 2, B], f32r, tag="msg2")
                incst = op.tile([B, 2 * L], f32, tag="incst")
                ps2 = ps2p.tile([B, 2 * L], f32, tag="ps2")
                nc.vector.tensor_tensor(
                    msg2[0:M, :, :],
                    ps1a[0:M, :, :],
                    bit[0:M, lsl].broadcast_to([M, GS // 2, B]),
                    add_op,
                )
                nc.vector.tensor_tensor(
                    msg2[M:, :, :],
                    ps1b[M:, :, :],
                    bit[M:, hsl2].broadcast_to([M, GS // 2, B]),
                    add_op,
                )
                nc.vector.tensor_tensor(
                    ott[:, 0 : GS // 2, :],
                    ps1a[:],
                    bit[:, lsl].broadcast_to([2 * M, GS // 2, B]),
                    add_op,
                )
                nc.vector.tensor_tensor(
                    ott[:, GS // 2 :, :],
                    ps1b[:],
                    bit[:, hsl2].broadcast_to([2 * M, GS // 2, B]),
                    add_op,
                )
                for q in range(GS // 2):
                    nc.tensor.matmul(
                        ps2[:],
                        msg2[:, q, :],
                        agt[:, q, :],
                        start=(q == 0),
                        stop=(q == GS // 2 - 1),
                    )
                nc.scalar.activation(incst[:], ps2[:], act_copy)
                nc.sync.dma_start(mo[:, gsl, :], ott[:])
                nc.sync.dma_start(io[:, g, :], incst[:])

            from collections import deque

            pending = deque()
            for g in range(ng):
                pending.append(emit_gemm1(g))
                if len(pending) > 2:
                    emit_epilogue(pending.popleft())
            while pending:
                emit_epilogue(pending.popleft())

    _split_excess_waits(nc)
    return nc


def _get_program(ng):
    if ng not in _prog_cache:
        _prog_cache[ng] = _build_program(ng)
    return _prog_cache[ng]


def _plan(src_idx, tgt_idx):
    """Deal edges to cores balanced per target; group into GS-slot runs."""
    order = np.argsort(tgt_idx, kind="stable")
    sorted_tgt = tgt_idx[order]
    bounds = np.searchsorted(sorted_tgt, np.arange(R + 1))
    core_slots = [[] for _ in range(NC)]
    core_gtgt = [[] for _ in range(NC)]
    for t in range(R):
        eds = order[bounds[t] : bounds[t + 1]]
        if len(eds) == 0:
            continue
        for k in range(NC):
            chunk = eds[(k + t) % NC :: NC]
            if len(chunk) == 0:
                continue
            npad = (-len(chunk)) % GS
            core_slots[k].extend(chunk.tolist())
            core_slots[k].extend([-1] * npad)
            core_gtgt[k].extend([t] * ((len(chunk) + npad) // GS))
    ng = max(len(gt) for gt in core_gtgt)
    for k in range(NC):
        pad_groups = ng - len(core_gtgt[k])
        core_slots[k].extend([-1] * (pad_groups * GS))
        core_gtgt[k].extend([-1] * pad_groups)
    slots = [np.asarray(s, np.int64) for s in core_slots]
    gtgt = [np.asarray(t, np.int64) for t in core_gtgt]
    return ng, slots, gtgt


def kernel(**inputs):
    source = np.ascontiguousarray(np.asarray(inputs["source"], _F32))
    mean_w = np.asarray(inputs["mean_w"], _F32)
    mean_b = np.asarray(inputs["mean_b"], _F32)
    logstd_w = np.asarray(inputs["logstd_w"], _F32)
    logstd_b = np.asarray(inputs["logstd_b"], _F32)
    add_w = np.asarray(inputs["add_w"], _F32)
    gain_w = np.asarray(inputs["gain_w"], _F32)
    src_idx = np.asarray(inputs["src_idx"]).astype(np.int64).ravel()
    tgt_idx = np.asarray(inputs["tgt_idx"]).astype(np.int64).ravel()

    ng, slots, gtgt = _plan(src_idx, tgt_idx)
    nslot = ng * GS
    srcT = np.ascontiguousarray(source.transpose(2, 1, 0))  # [S, R, B]

    in_maps = []
    for k in range(NC):
        sl = slots[k]
        real = sl >= 0
        jr = np.nonzero(real)[0]
        par = (jr % GS) < (GS // 2)
        je, jo = jr[par], jr[~par]
        ide, ido = sl[je], sl[jo]

        w1 = np.zeros((S, nslot, 2 * M), _F32)
        w1[:, je, 0:M] = mean_w[ide].transpose(2, 0, 1)
        w1[:, je, M:] = logstd_w[ide].transpose(2, 0, 1)
        w1[:, jo, 0:M] = logstd_w[ido].transpose(2, 0, 1)
        w1[:, jo, M:] = mean_w[ido].transpose(2, 0, 1)

        qe = (je // GS) * (GS // 2) + (je % GS)
        qo = (jo // GS) * (GS // 2) + (jo % GS) - GS // 2
        ag = np.zeros((2 * M, nslot // 2, 2 * L), _F32)
        ag[0:M, qe, 0:L] = add_w[ide].transpose(2, 0, 1)
        ag[0:M, qe, L:] = gain_w[ide].transpose(2, 0, 1)
        ag[M:, qo, 0:L] = add_w[ido].transpose(2, 0, 1)
        ag[M:, qo, L:] = gain_w[ido].transpose(2, 0, 1)

        sg = np.zeros((S, nslot, B), _F32)
        sg[:, jr, :] = srcT[:, src_idx[sl[jr]], :]

        bi = np.zeros((2 * M, nslot), _F32)
        bi[0:M, je] = mean_b[ide].T
        bi[M:, je] = logstd_b[ide].T
        bi[0:M, jo] = logstd_b[ido].T
        bi[M:, jo] = mean_b[ido].T
        in_maps.append({"w1": w1, "ag": ag, "sg": sg, "bi": bi})

    from concourse.bass_utils import run_bass_kernel_spmd

    nc = _get_program(ng)
    res = run_bass_kernel_spmd(nc, in_maps, list(range(NC)))

    inc_addT = np.zeros((R, B, L), _F32)
    inc_gainT = np.zeros((R, B, L), _F32)
    scat_mean = np.zeros((B, R * R, M), _F32)
    scat_logstd = np.zeros((B, R * R, M), _F32)
    for k in range(NC):
        out = res.results[k]
        mo = out["mo"]  # [2M, nslot, B]
        io = out["io"]  # [B, ng, 2L]
        gt = gtgt[k]
        valid = gt >= 0
        gsum = io[:, valid, :]  # [B, ngv, 2L]
        tv = gt[valid]
        np.add.at(inc_addT, tv, gsum[:, :, 0:L].transpose(1, 0, 2))
        np.add.at(inc_gainT, tv, gsum[:, :, L:].transpose(1, 0, 2))

        sl = slots[k]
        jr = np.nonzero(sl >= 0)[0]
        eids = sl[jr]
        flat = src_idx[eids] * R + tgt_idx[eids]
        even = (jr % GS) < (GS // 2)
        meanT = np.where(even[None, :, None], mo[0:M, jr, :], mo[M:, jr, :])
        logsT = np.where(even[None, :, None], mo[M:, jr, :], mo[0:M, jr, :])
        scat_mean[:, flat, :] = meanT.transpose(2, 1, 0)
        scat_logstd[:, flat, :] = logsT.transpose(2, 1, 0)

    inc_add = inc_addT.transpose(1, 0, 2).copy()
    inc_gain = inc_gainT.transpose(1, 0, 2).copy()
    scat_mean = scat_mean.reshape(B, R, R, M)
    scat_logstd = scat_logstd.reshape(B, R, R, M)
    return inc_add, inc_gain, scat_mean, scat_logstd, scat_mean.copy()
